# revision 1
# baseline (speedup 1.0000x reference)
"""DynamicConv1D Trainium2 kernel.

Reference computation (per batch b):
  dw = conv1d(x, W, pad=3) + b            # [O*I*K, T] dynamic weights
  dw = softmax(dw.reshape(O,I,K,T)/sqrt(K), axis=K)
  y[o,t] = sum_{i,k} x[i, t+k-3] * dw[o,i,k,t]

Sharding: 8 cores = 4 batches x 2 halves of O (16 out-channels each).
Each core gets x[b] plus its half of the (rearranged) conv weights and
computes y[b, half*16:(half+1)*16, :]. No collectives; the host scatters
inputs and concatenates outputs.

Per-core layout (t-tile = 128 positions on partitions):
  conv as matmul: dw[t, (k,o,i)] = sum_{(j,c)} X1[(j,c), t] * W'[(j,c), (k,o,i)]
    X1[(j,c), u] = x[c, u+j-3]  (im2col layout built host-side, bf16)
    ones row appended to X1 so the bias rides as an extra W' row;
    1/sqrt(K) is folded into W' and b on the host.
  softmax+einsum fused in SBUF (t on partitions, (k,o,i) on free):
    e = exp(dw) via ScalarE (PSUM -> SBUF, bf16)
    EX = e * x_unf[t,(k,i)]  (x broadcast over o via a stride-0 view; the
                              per-tile x_unf comes from two DMA transposes)
    den/num = sum_k {e, EX}  (batched bf16 pair-add tree; den lands f32)
    y[t,o] = sum_i num * (1/den)   (fast reciprocal + grouped reduce,
                                    batched over tile pairs)
"""

import numpy as np

B = 4
C = 32
K = 7
T = 4096
O_FULL = 32
OH = 16  # out-channels per core
PAD = 3
TT = 128  # t positions per tile (partition dim)
FREE = K * OH * C  # 3584, free index = k*512 + o*32 + i
SLAB = OH * C  # 512, one k-slab
CD1 = 128  # (j, c) rows for j=0..3
CD2 = 97  # (j, c) rows for j=4..6 plus ones row
CHUNK = 512  # psum chunk (1 bank); FREE = 7*CHUNK

_prog_cache = {}


def _build(t_len):
    """Build and compile the per-core Bass program for sequence length t_len."""
    import concourse.tile as tile
    from concourse import bacc, mybir

    nt = t_len // TT
    nc = bacc.Bacc("TRN2", target_bir_lowering=False, debug=False, num_devices=1)
    f32 = mybir.dt.float32
    f32r = mybir.dt.float32r
    bf16 = mybir.dt.bfloat16

    x1a_d = nc.dram_tensor("x1a", [CD1, t_len], bf16, kind="ExternalInput").ap()
    x1b_d = nc.dram_tensor("x1b", [CD2, t_len], bf16, kind="ExternalInput").ap()
    w1_d = nc.dram_tensor("wp1", [CD1, FREE], bf16, kind="ExternalInput").ap()
    w2_d = nc.dram_tensor("wp2", [CD2, FREE], bf16, kind="ExternalInput").ap()
    y_d = nc.dram_tensor("yout", [TT, nt * OH], f32, kind="ExternalOutput").ap()

    with tile.TileContext(nc) as tc:
        with (
            tc.tile_pool(name="const", bufs=1) as cpool,
            tc.tile_pool(name="x2p", bufs=4) as x2pool,
            tc.tile_pool(name="ep", bufs=4) as epool,
            tc.tile_pool(name="tree", bufs=2) as tpool,
            tc.tile_pool(name="small", bufs=3) as spool,
            tc.tile_pool(name="psum", bufs=8, space="PSUM") as ppool,
        ):
            # --- constants: im2col X1 (bf16, shipped pre-cast), weights ---
            x1a_bf = cpool.tile([CD1, t_len], bf16, tag="x1abf")
            x1b_bf = cpool.tile([CD2, t_len], bf16, tag="x1bbf")
            w1_bf = cpool.tile([CD1, FREE], bf16, tag="w1bf")
            w2_bf = cpool.tile([CD2, FREE], bf16, tag="w2bf")
            y_sb = cpool.tile([TT, nt * OH], f32, tag="ysb")

            # im2col X1 is pre-built on the host (rows j*32+c = x[c, u+j-3],
            # zero-padded, ones row last for the bias). Split loads across the
            # sync and gpsimd DMA queues, first-needed columns first.
            h = t_len // 2
            hf = FREE // 2
            nc.sync.dma_start(x1a_bf[:, 0:h], x1a_d[:, 0:h])
            nc.gpsimd.dma_start(x1b_bf[:, 0:h], x1b_d[:, 0:h])
            nc.sync.dma_start(w1_bf[:, 0:hf], w1_d[:, 0:hf])
            nc.gpsimd.dma_start(w2_bf[:, 0:hf], w2_d[:, 0:hf])
            nc.sync.dma_start(w1_bf[:, hf:], w1_d[:, hf:])
            nc.gpsimd.dma_start(w2_bf[:, hf:], w2_d[:, hf:])
            nc.sync.dma_start(x1a_bf[:, h:], x1a_d[:, h:])
            nc.gpsimd.dma_start(x1b_bf[:, h:], x1b_d[:, h:])

            nchunks = FREE // CHUNK  # 7

            for tt in range(nt):
                t0 = tt * TT
                # x_unf for this tile: X2[tp, k*32+i] = x[i, t0+tp+k-3]
                x2 = x2pool.tile([TT, K * C], bf16, tag="x2")
                nc.sync.dma_start_transpose(x2[:, 0:CD1], x1a_bf[:, t0 : t0 + TT])
                nc.sync.dma_start_transpose(
                    x2[:, CD1 : K * C], x1b_bf[0 : CD2 - 1, t0 : t0 + TT]
                )

                # eex holds e (exp of dw) and EX (e * x_unf) side by side so
                # the den/num k-sum trees batch into single wide DVE ops.
                eex = epool.tile([TT, 2, FREE], bf16, tag="eex")
                e = eex[:, 0]
                ex = eex[:, 1]
                # Interleave the two contraction halves per chunk so each
                # PSUM bank finishes (and ScalarE can drain it) as early as
                # possible; weights reload per-matmul either way.
                for ci in range(nchunks):
                    pc = ppool.tile([TT, CHUNK], f32, tag="pc", name="pc")
                    cs = slice(ci * CHUNK, (ci + 1) * CHUNK)
                    nc.tensor.matmul(
                        pc[:], x1a_bf[:, t0 : t0 + TT], w1_bf[:, cs],
                        start=True, stop=False,
                    )
                    nc.tensor.matmul(
                        pc[:], x1b_bf[:, t0 : t0 + TT], w2_bf[:, cs],
                        start=False, stop=True,
                    )
                    nc.scalar.activation(
                        e[:, cs], pc[:], mybir.ActivationFunctionType.Exp
                    )

                # EX = e * x_unf broadcast over o
                e4 = e.rearrange("p (k o i) -> p k o i", k=K, o=OH)
                x24 = (
                    x2[:]
                    .rearrange("p (k i) -> p k i", k=K)
                    .unsqueeze(2)
                    .broadcast_to([TT, K, OH, C])
                )
                ex4 = ex.rearrange("p (k o i) -> p k o i", k=K, o=OH)
                nc.vector.tensor_mul(ex4, e4, x24)

                # k-sum trees for den (over e) and num (over EX), batched as
                # one wide op per tree level via the [TT, 2, ...] eex view.
                # v0..v6 are the k-slabs: lvl1 (v0+v1, v2+v3, v4+v5), then
                # d=(a+b), g=(c+v6), dennum=d+g.
                pairs = eex[:, :, 0 : 6 * SLAB].rearrange(
                    "p s (x k q) -> p s x k q", x=3, k=2
                )
                t1 = tpool.tile([TT, 2, 3, SLAB], bf16, tag="t1")
                nc.vector.tensor_add(t1[:], pairs[:, :, :, 0], pairs[:, :, :, 1])
                t2 = tpool.tile([TT, 2, SLAB], bf16, tag="t2")
                nc.vector.tensor_add(t2[:], t1[:, :, 0], t1[:, :, 1])
                t3 = tpool.tile([TT, 2, SLAB], bf16, tag="t3")
                sl6 = eex[:].rearrange("p s (k q) -> p s k q", k=K)[:, :, 6]
                nc.vector.tensor_add(t3[:], t1[:, :, 2], sl6)
                if tt % 2 == 0:
                    dn2 = spool.tile([TT, 2, SLAB], bf16, tag="dn2")
                    denf = spool.tile([TT, 2, SLAB], f32, tag="denf")
                # den goes straight to f32 (reciprocal needs it); num stays
                # bf16.
                nc.vector.tensor_add(denf[:, tt % 2], t2[:, 0], t3[:, 0])
                nc.vector.tensor_add(dn2[:, tt % 2], t2[:, 1], t3[:, 1])

                if tt % 2 == 1:
                    # softmax tail for the tile pair: 1/den, then
                    # y[t,o] = sum_i num * r
                    r = spool.tile([TT, 2, SLAB], f32, tag="r")
                    nc.vector.reciprocal_approx_fast(out=r[:], in_=denf[:])
                    y1 = spool.tile([TT, 2, SLAB], f32, tag="y1")
                    nc.vector.tensor_mul(y1[:], dn2[:], r[:])
                    nc.vector.tensor_reduce(
                        y_sb[:, (tt - 1) * OH : (tt + 1) * OH],
                        y1[:].rearrange("p u (o i) -> p u o i", o=OH),
                        axis=mybir.AxisListType.X,
                        op=mybir.AluOpType.add,
                    )

                if (tt + 1) % 8 == 0 or tt == nt - 1:
                    g0 = (tt // 8) * 8 * OH
                    nc.gpsimd.dma_start(
                        y_d[:, g0 : (tt + 1) * OH], y_sb[:, g0 : (tt + 1) * OH]
                    )

    nc.compile()
    return nc


def _prep_inputs(x, W, b):
    """Host-side scatter: per-core input dicts (pure layout/slicing)."""
    import ml_dtypes

    bf = ml_dtypes.bfloat16
    scale = np.float32(1.0 / np.sqrt(K))
    halves = []
    for h in range(2):
        Wh = W[h * OH * C * K : (h + 1) * OH * C * K]  # [OH*C*K, C, K]
        # rows (j,c) -> j*32+c ; cols (k,o,i) -> k*512 + o*32 + i
        Wp = (
            Wh.reshape(OH, C, K, C, K).transpose(4, 3, 2, 0, 1).reshape(K * C, FREE)
            * scale
        )
        bh = (
            b[h * OH * C * K : (h + 1) * OH * C * K]
            .reshape(OH, C, K)
            .transpose(2, 0, 1)
            .reshape(FREE)
            * scale
        )
        w1 = np.ascontiguousarray(Wp[:CD1])
        w2 = np.ascontiguousarray(
            np.concatenate([Wp[CD1:], bh[None, :]], axis=0)
        )
        halves.append((w1.astype(bf), w2.astype(bf)))

    t_len = x.shape[-1]
    x1s = []
    for bi in range(B):
        xp = np.zeros((C, t_len + 2 * PAD), dtype=np.float32)
        xp[:, PAD : PAD + t_len] = x[bi]
        x1a = np.empty((CD1, t_len), dtype=np.float32)
        x1b = np.empty((CD2, t_len), dtype=np.float32)
        for j in range(K):
            tgt, r0 = (x1a, j * C) if j < 4 else (x1b, (j - 4) * C)
            tgt[r0 : r0 + C] = xp[:, j : j + t_len]
        x1b[CD2 - 1] = 1.0
        x1s.append((x1a.astype(bf), x1b.astype(bf)))

    in_maps = []
    for core in range(8):
        bi, h = divmod(core, 2)
        w1, w2 = halves[h]
        x1a, x1b = x1s[bi]
        in_maps.append({"x1a": x1a, "x1b": x1b, "wp1": w1, "wp2": w2})
    return in_maps


def _assemble(results, t_len):
    """Gather per-core [TT, nt*OH] outputs into [B, O_FULL, t_len]."""
    nt = t_len // TT
    y = np.empty((B, O_FULL, t_len), dtype=np.float32)
    for core, res in enumerate(results):
        bi, h = divmod(core, 2)
        arr = res["yout"].reshape(TT, nt, OH)  # [tp, tt, o]
        y[bi, h * OH : (h + 1) * OH, :] = arr.transpose(2, 1, 0).reshape(OH, t_len)
    return y


def _run(x, W, b, trace=False, trace_cores=None):
    from concourse.bass_utils import run_bass_kernel_spmd
    from concourse.bass_interp import get_hw_module

    t_len = x.shape[-1]
    key = ("prog", t_len)
    if key not in _prog_cache:
        nc = _build(t_len)
        nc.m = get_hw_module(nc.m)
        _prog_cache[key] = nc
    nc = _prog_cache[key]

    in_maps = _prep_inputs(x, W, b)
    res = run_bass_kernel_spmd(
        nc,
        in_maps,
        core_ids=list(range(8)),
        trace=trace,
        trace_cores=trace_cores,
    )
    return _assemble(res.results, t_len), res


def kernel(x, W, b):
    y, _ = _run(np.asarray(x), np.asarray(W), np.asarray(b))
    return y

